# revision 13
# baseline (speedup 1.0000x reference)
"""
CSAM (channel self-attention) Trainium2 Bass kernel, v2.

Computation (per batch b):
    q = x[b].reshape(C, N)                 # C=64, N=192*192=36864
    E = q @ q.T                            # [64, 64] channel gram
    A = softmax(rowmax(E) - E) over rows   # == softmax(-E) stabilized by rowmin
    out = A @ q
    res = x * (gamma * out) + x

Sharding: data-parallel over batch. 8 cores x 4 batches each; identical NEFF per
core on its own batch slice (no collectives).

v2 design (vs v1): PE was the bottleneck (transpose-mode MMs don't warm the HAM
clock; avg MM 269ns). Changes:
 - layout p = h*64 + c (x[b].rearrange("c (h j) -> (h c) j")): each partition is
   a contiguous DRAM run; transposed chunks get dense per-h channel halves, so
   energy matmuls need no parity reorder and W = blockdiag(A^T, A^T).
 - loads cast fp32->bf16 during SWDGE DMA: no staging buffers, no cast pass.
 - transposes are regular matmuls against a bf16 identity (FWL + warm clock).
 - E accumulated in one PSUM region over 288 [128,64]x[128,64] matmuls.
 - epilogue res = (po + 1) * q16 (gamma folded into A) on DVE; PSUM->SBUF
   transpose copies on ACT; res stored fp32 via HWDGE.
"""

import sys

sys.path.insert(0, "/opt/trn_rl_repo")

import numpy as np

import concourse.bass as bass
import concourse.bacc as bacc
import concourse.tile as tile
from concourse import mybir
from concourse.bass_utils import run_bass_kernel_spmd
from concourse.masks import make_identity

N_CORES = 8
B_FULL, C, H, W = 32, 64, 192, 192
N = H * W                  # 36864
NH = N // 2                # 18432 flat free width
B_PER = B_FULL // N_CORES  # 4 batches per core
NHH = NH // 2              # 9216 half-batch width
NCH = NH // 128            # 144 transpose chunks per batch
TG = 4                     # chunks per transpose group (one PSUM bank)
NTG = NCH // TG            # 36 groups per batch
OC = 512                   # out-matmul chunk (one PSUM bank)
NOC = NH // OC             # 36 out chunks per batch
RW = NH // 4               # 4608 store / load-quarter width

f32 = mybir.dt.float32
bf16 = mybir.dt.bfloat16

_CACHED_NC = None


def _build():
    nc = bacc.Bacc("TRN2", target_bir_lowering=False, debug=False)
    x_d = nc.dram_tensor("x", [B_PER, C, N], f32, kind="ExternalInput").ap()
    g_d = nc.dram_tensor("gamma", [1], f32, kind="ExternalInput").ap()
    o_d = nc.dram_tensor("out", [B_PER, C, N], f32, kind="ExternalOutput").ap()

    with tile.TileContext(nc) as tc:
        with (
            tc.tile_pool(name="const", bufs=1) as constp,
            tc.tile_pool(name="q16", bufs=4) as q16p,
            tc.tile_pool(name="xst", bufs=2) as xstp,
            tc.tile_pool(name="qT", bufs=1) as qTp,
            tc.tile_pool(name="res", bufs=2) as resp,
            tc.tile_pool(name="sm", bufs=2) as smp,
            tc.tile_pool(name="w2", bufs=2) as w2p,
            tc.tile_pool(name="psT", bufs=3, space="PSUM") as psTp,
            tc.tile_pool(name="psE", bufs=2, space="PSUM") as psEp,
            tc.tile_pool(name="psO", bufs=2, space="PSUM") as psOp,
            tc.tile_pool(name="psA", bufs=1, space="PSUM") as psAp,
        ):
            ident16 = constp.tile([128, 128], bf16)
            make_identity(nc, ident16[:])
            g1 = constp.tile([1, 1], f32)
            nc.sync.dma_start(g1[:], g_d[None, :])
            gb = constp.tile([128, 1], f32)
            nc.gpsimd.partition_broadcast(gb[:], g1[:])

            def load(b):
                # p = h*64 + c; each partition one contiguous 73728B DRAM run.
                # DMA pairs stream order [h][c][j] <-> [p][j], so p = h*64+c.
                # HWDGE fp32 loads into staging, then cast to bf16 on-chip
                # (SWDGE cast-DMA measured ~39 GB/s -- unusable).
                xb = x_d[b].rearrange("c (h j) -> h c j", h=2)
                halves = []
                for hh in range(2):
                    q16h = q16p.tile([128, NHH], bf16, tag="q16", name="q16h")
                    for qq in range(2):
                        xst = xstp.tile([128, RW], f32, tag="xst", name="xst")
                        off = hh * NHH + qq * RW
                        nc.sync.dma_start(xst[:], xb[:, :, off : off + RW])
                        dst = q16h[:, qq * RW : (qq + 1) * RW]
                        # DVE is the only fast f32->bf16 caster (2x_2P);
                        # ACT copy is 4us and GPSIMD CAST 15.5us per quarter
                        nc.vector.tensor_copy(dst, xst[:])
                    halves.append(q16h)
                return halves

            def phase1(b, q16h):
                """Transpose + energy accumulation; yields after each group."""
                qT = qTp.tile([128, NCH, 128], bf16, tag="qT")
                E_ps = psEp.tile([C, C], f32, tag="E")

                def emit_E(gi):
                    for i in range(TG):
                        t = gi * TG + i
                        for h in range(2):
                            lr = qT[:, t, 64 * h : 64 * h + 64]
                            nc.tensor.matmul(
                                E_ps[:],
                                lr,
                                lr,
                                start=(t == 0 and h == 0),
                                stop=(t == NCH - 1 and h == 1),
                            )

                for gi in range(NTG):
                    pq = psTp.tile([128, TG, 128], f32, tag="pq")
                    for i in range(TG):
                        t = gi * TG + i
                        src = q16h[t // (NCH // 2)]
                        tt = t % (NCH // 2)
                        nc.tensor.matmul(
                            pq[:, i, :],
                            src[:, tt * 128 : (tt + 1) * 128],
                            ident16[:],
                            start=True,
                            stop=True,
                        )
                    nc.scalar.copy(qT[:, gi * TG : (gi + 1) * TG, :], pq[:])
                    # energy matmuls lag two groups so the copy always lands
                    # before PE needs it (no PE stall waiting on ACT)
                    if gi > 1:
                        emit_E(gi - 2)
                    yield gi
                emit_E(NTG - 2)
                emit_E(NTG - 1)
                phase1.E_ps = E_ps

            def softmax_W2(E_ps):
                E = smp.tile([C, C], f32, tag="E")
                nc.vector.tensor_copy(E[:], E_ps[:])
                m = smp.tile([C, 1], f32, tag="m")
                nc.vector.tensor_reduce(
                    m[:], E[:], axis=mybir.AxisListType.X, op=mybir.AluOpType.min
                )
                texp = smp.tile([C, C], f32, tag="texp")
                Z = smp.tile([C, 1], f32, tag="Z")
                nc.scalar.activation(
                    texp[:],
                    E[:],
                    mybir.ActivationFunctionType.Exp,
                    bias=m[:],
                    scale=-1.0,
                    accum_out=Z[:],
                )
                r = smp.tile([C, 1], f32, tag="r")
                nc.vector.reciprocal(r[:], Z[:])
                # fold gamma into A: epilogue becomes res = (out + 1) * x
                rg = smp.tile([C, 1], f32, tag="rg")
                nc.vector.tensor_tensor(
                    rg[:], r[:], gb[0:64, :], mybir.AluOpType.mult
                )
                A16 = smp.tile([C, C], bf16, tag="A16")
                nc.vector.tensor_scalar_mul(A16[:], texp[:], rg[:])
                # W2 = blockdiag(A^T, A^T) : po[h*64+c] = sum_d A[c,d] q[h*64+d]
                psA = psAp.tile([128, C], f32, tag="psA")
                nc.tensor.matmul(
                    psA[0:64, :], A16[:], ident16[0:64, 0:64], start=True, stop=True
                )
                nc.tensor.matmul(
                    psA[64:128, :], A16[:], ident16[0:64, 0:64], start=True, stop=True
                )
                W2 = w2p.tile([128, 128], bf16, tag="W2")
                nc.gpsimd.memset(W2[:], 0.0)
                nc.scalar.copy(W2[0:64, 0:64], psA[0:64, :])
                nc.vector.tensor_copy(W2[64:128, 64:128], psA[64:128, :])
                return W2

            def phase2_chunk(b, q16h, W2, res_holder, k):
                ob = o_d[b].rearrange("c (h j) -> h c j", h=2)
                per = RW // OC  # out chunks per res quarter
                if k % per == 0:
                    res_holder[0] = resp.tile(
                        [128, RW], f32, tag="res", name="res"
                    )
                res = res_holder[0]
                src = q16h[k // (NOC // 2)]
                kk = k % (NOC // 2)
                po = psOp.tile([128, OC], f32, tag="po")
                nc.tensor.matmul(
                    po[:],
                    W2[:],
                    src[:, kk * OC : (kk + 1) * OC],
                    start=True,
                    stop=True,
                )
                off = (k % per) * OC
                nc.vector.scalar_tensor_tensor(
                    res[:, off : off + OC],
                    po[:],
                    1.0,
                    src[:, kk * OC : (kk + 1) * OC],
                    mybir.AluOpType.add,
                    mybir.AluOpType.mult,
                )
                if k % per == per - 1:
                    quarter = k // per
                    # stores on the ACT HWDGE ring; loads own the SP ring
                    nc.scalar.dma_start(
                        ob[:, :, quarter * RW : (quarter + 1) * RW], res[:]
                    )

            prev = None
            pending = load(0)
            for b in range(B_PER):
                q16h = pending
                for gi in phase1(b, q16h):
                    if gi == 18 and b + 1 < B_PER:
                        pending = load(b + 1)
                    if prev is not None:
                        phase2_chunk(*prev, gi)
                W2 = softmax_W2(phase1.E_ps)
                prev = (b, q16h, W2, [None])
            for k in range(NOC):
                phase2_chunk(*prev, k)

    nc.compile()
    return nc


def _get_nc():
    global _CACHED_NC
    if _CACHED_NC is None:
        _CACHED_NC = _build()
    return _CACHED_NC


def kernel(x: np.ndarray, gamma: np.ndarray, _collect=None) -> np.ndarray:
    assert x.shape == (B_FULL, C, H, W) and x.dtype == np.float32
    nc = _get_nc()
    xr = np.ascontiguousarray(x.reshape(B_FULL, C, N), dtype=np.float32)
    gamma = np.ascontiguousarray(gamma, dtype=np.float32)
    in_maps = [
        {"x": xr[i * B_PER : (i + 1) * B_PER], "gamma": gamma}
        for i in range(N_CORES)
    ]
    r = run_bass_kernel_spmd(nc, in_maps, core_ids=list(range(N_CORES)))
    if _collect is not None:
        _collect.append(r)
    out = np.concatenate([r.results[i]["out"] for i in range(N_CORES)], axis=0)
    return out.reshape(B_FULL, C, H, W).astype(np.float32)


# revision 19
# speedup vs baseline: 5.7746x; 5.7746x over previous
"""
CSAM (channel self-attention) Trainium2 Bass kernel, v5.

Computation (per batch b):
    q = x[b].reshape(C, N)                 # C=64, N=192*192=36864
    E = q @ q.T                            # [64, 64] channel gram
    A = softmax(rowmax(E) - E) over rows   # == softmax(-E) stabilized by rowmin
    out = A @ q
    res = x * (gamma * out) + x

Sharding: data-parallel over batch. 8 cores x 4 batches each; identical NEFF per
core on its own batch slice (no collectives).

Design notes (evidence from neuron-profile traces):
 - DMA layout p = 2c + h ("(c h) j" grouping): 2D DRAM APs spray across all 16
   SDMA engines (a 3D [2,64,w] AP measured 54 GB/s = 2 engines; SWDGE cast-DMA
   measured 39 GB/s -- both unusable). Loads on the SP HWDGE ring, stores on
   the ACT ring.
 - x is cast fp32->bf16 once on DVE (2x_2P, the only fast caster: ACT copy is
   4us/quarter, GPSIMD CAST 15.5us/quarter).
 - transposes are regular bf16 matmuls against an identity (transpose-mode PE
   ops never warm the HAM clock; regular matmuls run 2.4 GHz + FWL).
 - energy: one full-width [128,128] matmul per 128-column chunk accumulating
   E2[p,p'] (cross-parity terms included); E[c,d] = E2[2c,2d] + E2[2c+1,2d+1]
   extracted once per batch with selector matmuls (Se/So).
 - W = kron(A^T, I2) built on-chip (K2e/K2o selectors); gamma folded into A so
   the epilogue is res = (out + 1) * q16 (bf16 q; ~1.4e-3 rel err, tol 2e-2).
 - PSUM->SBUF transpose copies on ACT, unstrided [128,512]; epilogue
   scalar_tensor_tensor on DVE; energy matmuls lag 2 groups behind the copies.
"""

import sys

sys.path.insert(0, "/opt/trn_rl_repo")

import numpy as np

import concourse.bass as bass
import concourse.bacc as bacc
import concourse.tile as tile
from concourse import mybir
from concourse.bass_utils import run_bass_kernel_spmd
from concourse.masks import make_identity

N_CORES = 8
B_FULL, C, H, W = 32, 64, 192, 192
N = H * W                  # 36864
NH = N // 2                # 18432 flat free width
B_PER = B_FULL // N_CORES  # 4 batches per core
NHH = NH // 2              # 9216 half-batch width
NCH = NH // 128            # 144 transpose chunks per batch
TG = 4                     # chunks per transpose group (one PSUM bank)
NTG = NCH // TG            # 36 groups per batch
OC = 512                   # out-matmul chunk (one PSUM bank)
NOC = NH // OC             # 36 out chunks per batch
RW = NH // 4               # 4608 store / load-quarter width

f32 = mybir.dt.float32
bf16 = mybir.dt.bfloat16

_CACHED_NC = None


def _build():
    nc = bacc.Bacc("TRN2", target_bir_lowering=False, debug=False)
    x_d = nc.dram_tensor("x", [B_PER, C, N], f32, kind="ExternalInput").ap()
    g_d = nc.dram_tensor("gamma", [1], f32, kind="ExternalInput").ap()
    o_d = nc.dram_tensor("out", [B_PER, C, N], f32, kind="ExternalOutput").ap()

    with tile.TileContext(nc) as tc:
        with (
            tc.tile_pool(name="const", bufs=1) as constp,
            tc.tile_pool(name="q16", bufs=4) as q16p,
            tc.tile_pool(name="xst", bufs=2) as xstp,
            tc.tile_pool(name="qT", bufs=1) as qTp,
            tc.tile_pool(name="res", bufs=2) as resp,
            tc.tile_pool(name="sm", bufs=2) as smp,
            tc.tile_pool(name="w2", bufs=2) as w2p,
            tc.tile_pool(name="psT", bufs=2, space="PSUM") as psTp,
            tc.tile_pool(name="psE", bufs=1, space="PSUM") as psEp,
            tc.tile_pool(name="psO", bufs=2, space="PSUM") as psOp,
            tc.tile_pool(name="psA", bufs=1, space="PSUM") as psAp,
        ):
            ident16 = constp.tile([128, 128], bf16)
            make_identity(nc, ident16[:])
            g1 = constp.tile([1, 1], f32)
            nc.sync.dma_start(g1[:], g_d[None, :])
            gb = constp.tile([128, 1], f32)
            nc.gpsimd.partition_broadcast(gb[:], g1[:])

            # selector constants
            # K2e[d, m] = 1 iff m == 2d ; K2o[d, m] = 1 iff m == 2d+1
            K2e = constp.tile([64, 128], bf16)
            nc.gpsimd.memset(K2e[:], 0.0)
            nc.gpsimd.affine_select(
                out=K2e[:], in_=K2e[:],
                compare_op=mybir.AluOpType.not_equal,
                fill=1.0, base=0, pattern=[[-1, 128]], channel_multiplier=2,
            )
            K2o = constp.tile([64, 128], bf16)
            nc.gpsimd.memset(K2o[:], 0.0)
            nc.gpsimd.affine_select(
                out=K2o[:], in_=K2o[:],
                compare_op=mybir.AluOpType.not_equal,
                fill=1.0, base=1, pattern=[[-1, 128]], channel_multiplier=2,
            )
            # Se[p, d] = 1 iff p == 2d ; So[p, d] = 1 iff p == 2d+1
            Se = constp.tile([128, 64], bf16)
            nc.gpsimd.memset(Se[:], 0.0)
            nc.gpsimd.affine_select(
                out=Se[:], in_=Se[:],
                compare_op=mybir.AluOpType.not_equal,
                fill=1.0, base=0, pattern=[[2, 64]], channel_multiplier=-1,
            )
            So = constp.tile([128, 64], bf16)
            nc.gpsimd.memset(So[:], 0.0)
            nc.gpsimd.affine_select(
                out=So[:], in_=So[:],
                compare_op=mybir.AluOpType.not_equal,
                fill=1.0, base=1, pattern=[[2, 64]], channel_multiplier=-1,
            )

            def load(b):
                # p = 2c + h: adjacent grouping -> 2D DRAM AP, 16-engine DMA
                xb = x_d[b].rearrange("c (h j) -> (c h) j", h=2)
                halves = []
                for hh in range(2):
                    q16h = q16p.tile([128, NHH], bf16, tag="q16", name="q16h")
                    for qq in range(2):
                        xst = xstp.tile([128, RW], f32, tag="xst", name="xst")
                        off = hh * NHH + qq * RW
                        nc.sync.dma_start(xst[:], xb[:, off : off + RW])
                        nc.vector.tensor_copy(
                            q16h[:, qq * RW : (qq + 1) * RW], xst[:]
                        )
                    halves.append(q16h)
                return halves

            def phase1(b, q16h):
                """Transpose + energy accumulation; yields after each group."""
                qT = qTp.tile([128, NCH, 128], bf16, tag="qT")
                E2_ps = psEp.tile([128, 128], f32, tag="E2")

                def emit_E(gi):
                    for i in range(TG):
                        t = gi * TG + i
                        nc.tensor.matmul(
                            E2_ps[:],
                            qT[:, t, :],
                            qT[:, t, :],
                            start=(t == 0),
                            stop=(t == NCH - 1),
                        )

                for gi in range(NTG):
                    pq = psTp.tile([128, TG, 128], f32, tag="pq")
                    for i in range(TG):
                        t = gi * TG + i
                        src = q16h[t // (NCH // 2)]
                        tt = t % (NCH // 2)
                        nc.tensor.matmul(
                            pq[:, i, :],
                            src[:, tt * 128 : (tt + 1) * 128],
                            ident16[:],
                            start=True,
                            stop=True,
                        )
                    nc.scalar.copy(qT[:, gi * TG : (gi + 1) * TG, :], pq[:])
                    # energy matmuls lag two groups so the copy always lands
                    # before PE needs it (no PE stall waiting on ACT)
                    if gi > 1:
                        emit_E(gi - 2)
                    yield gi
                emit_E(NTG - 2)
                emit_E(NTG - 1)
                phase1.E2_ps = E2_ps

            def softmax_W(E2_ps):
                # E[c,d] = E2[2c,2d] + E2[2c+1,2d+1] via selector matmuls
                E2sb = smp.tile([128, 128], bf16, tag="E2sb")
                nc.vector.tensor_copy(E2sb[:], E2_ps[:])
                # one PSUM bank shared by tmp (cols 0:2) and E (col 2)
                tE = psAp.tile([128, 3, 64], f32, tag="tE")
                tmp_ps = tE[:, 0:2, :]
                # tmp_e[m, d] = E2[2d, m] ; tmp_o[m, d] = E2[2d+1, m]
                nc.tensor.matmul(tmp_ps[:, 0, :], E2sb[:], Se[:], start=True, stop=True)
                nc.tensor.matmul(tmp_ps[:, 1, :], E2sb[:], So[:], start=True, stop=True)
                tmp_sb = smp.tile([128, 2, 64], bf16, tag="tmp_sb")
                nc.scalar.copy(tmp_sb[:], tmp_ps[:])
                E_ps = tE[0:64, 2, :]
                # E[c,d] = tmp_e[2c, d] + tmp_o[2c+1, d]
                nc.tensor.matmul(E_ps[:], Se[:], tmp_sb[:, 0, :], start=True, stop=False)
                nc.tensor.matmul(E_ps[:], So[:], tmp_sb[:, 1, :], start=False, stop=True)

                E = smp.tile([C, C], f32, tag="E")
                nc.vector.tensor_copy(E[:], E_ps[:])
                m = smp.tile([C, 1], f32, tag="m")
                nc.vector.tensor_reduce(
                    m[:], E[:], axis=mybir.AxisListType.X, op=mybir.AluOpType.min
                )
                texp = smp.tile([C, C], f32, tag="texp")
                Z = smp.tile([C, 1], f32, tag="Z")
                nc.scalar.activation(
                    texp[:],
                    E[:],
                    mybir.ActivationFunctionType.Exp,
                    bias=m[:],
                    scale=-1.0,
                    accum_out=Z[:],
                )
                r = smp.tile([C, 1], f32, tag="r")
                nc.vector.reciprocal(r[:], Z[:])
                # fold gamma into A: epilogue becomes res = (out + 1) * x
                rg = smp.tile([C, 1], f32, tag="rg")
                nc.vector.tensor_tensor(
                    rg[:], r[:], gb[0:64, :], mybir.AluOpType.mult
                )
                A16 = smp.tile([C, C], bf16, tag="A16")
                nc.vector.tensor_scalar_mul(A16[:], texp[:], rg[:])
                # W = kron(A^T, I2): W[2d+h, 2c+h] = A[c, d]
                # one PSUM bank shared by Zp (cols 0:2) and Wp (col 2)
                ZW = psAp.tile([128, 3, 128], f32, tag="ZW")
                Zp = ZW[0:64, 0:2, :]
                nc.tensor.matmul(Zp[:, 0, :], A16[:], K2e[:], start=True, stop=True)
                nc.tensor.matmul(Zp[:, 1, :], A16[:], K2o[:], start=True, stop=True)
                Zsb = smp.tile([C, 2, 128], bf16, tag="Zsb")
                nc.scalar.copy(Zsb[:], Zp[:])
                Wp = ZW[:, 2, :]
                nc.tensor.matmul(Wp[:], K2e[:], Zsb[:, 0, :], start=True, stop=False)
                nc.tensor.matmul(Wp[:], K2o[:], Zsb[:, 1, :], start=False, stop=True)
                Wsb = w2p.tile([128, 128], bf16, tag="Wsb")
                nc.scalar.copy(Wsb[:], Wp[:])
                return Wsb

            def phase2_chunk(b, q16h, Wsb, res_holder, k):
                ob = o_d[b].rearrange("c (h j) -> (c h) j", h=2)
                per = RW // OC  # out chunks per res quarter
                if k % per == 0:
                    res_holder[0] = resp.tile(
                        [128, RW], f32, tag="res", name="res"
                    )
                res = res_holder[0]
                src = q16h[k // (NOC // 2)]
                kk = k % (NOC // 2)
                po = psOp.tile([128, OC], f32, tag="po")
                nc.tensor.matmul(
                    po[:],
                    Wsb[:],
                    src[:, kk * OC : (kk + 1) * OC],
                    start=True,
                    stop=True,
                )
                off = (k % per) * OC
                nc.vector.scalar_tensor_tensor(
                    res[:, off : off + OC],
                    po[:],
                    1.0,
                    src[:, kk * OC : (kk + 1) * OC],
                    mybir.AluOpType.add,
                    mybir.AluOpType.mult,
                )
                if k % per == per - 1:
                    quarter = k // per
                    # stores on the ACT HWDGE ring; loads own the SP ring
                    nc.scalar.dma_start(
                        ob[:, quarter * RW : (quarter + 1) * RW], res[:]
                    )

            prev = None
            pending = load(0)
            for b in range(B_PER):
                q16h = pending
                for gi in phase1(b, q16h):
                    if gi == 18 and b + 1 < B_PER:
                        pending = load(b + 1)
                    if prev is not None:
                        phase2_chunk(*prev, gi)
                Wsb = softmax_W(phase1.E2_ps)
                prev = (b, q16h, Wsb, [None])
            for k in range(NOC):
                phase2_chunk(*prev, k)

    nc.compile()
    return nc


def _get_nc():
    global _CACHED_NC
    if _CACHED_NC is None:
        _CACHED_NC = _build()
    return _CACHED_NC


def kernel(x: np.ndarray, gamma: np.ndarray, _collect=None) -> np.ndarray:
    assert x.shape == (B_FULL, C, H, W) and x.dtype == np.float32
    nc = _get_nc()
    xr = np.ascontiguousarray(x.reshape(B_FULL, C, N), dtype=np.float32)
    gamma = np.ascontiguousarray(gamma, dtype=np.float32)
    in_maps = [
        {"x": xr[i * B_PER : (i + 1) * B_PER], "gamma": gamma}
        for i in range(N_CORES)
    ]
    r = run_bass_kernel_spmd(nc, in_maps, core_ids=list(range(N_CORES)))
    if _collect is not None:
        _collect.append(r)
    out = np.concatenate([r.results[i]["out"] for i in range(N_CORES)], axis=0)
    return out.reshape(B_FULL, C, H, W).astype(np.float32)
